# revision 25
# baseline (speedup 1.0000x reference)
"""Distributed BatchSpectralLoss kernel for Trainium2 (8 NeuronCores).

Computes sum of top-k squared singular values of x (= top-k eigenvalues of
the Gram matrix G = x^T x) for x of shape (8192, 4096), k small (k=1).

Algorithm — implicit block Krylov on x (G is never formed):
  Host: scale x by 1/sqrt(C) with C = 3*||x||_F^2/N so lamhat_1 = O(1) in
  bf16, and draw per-chain random start blocks Omega [4096, b].
  Device, per core r (bf16 matmuls, fp32 PSUM; r owns 1024 rows of x):
    SBUF-resident x slices: xrT = x[rows_r,:]^T (lhsT for U = x_r @ Y,
    loaded as 8 per-m-tile pieces striped over two DMA queues so the first
    U matmuls start ~8us in) and xrN = x[rows_r,:] (lhsT for the partial
    Y-update, striped over the vector+gpsimd queues).  A short PE warmup
    burst runs during the loads to beat the HAM cold-clock ramp.
    Per application t (q_c per chain, chains phase-shifted so one chain's
    matmuls hide the other's AllReduce; the gpsimd queue carries ONLY the
    collective triggers so ARs stream back-to-back):
      U_t[rows_r] = x[rows_r,:] @ Y_t           [1024, b]  (stays in SBUF)
      Ypart       = x[rows_r,:]^T @ U_t[rows_r] [4096, b]  (pi-major)
      AllReduce(add, bf16) Ypart -> Y_{t+1} full on every core.
    After the last level one extra U_{q_c} = x_r @ Y_{q_c}.
    The U_t sequence is a Krylov sequence of H = xhat xhat^T (same nonzero
    eigenvalues as Ghat), and its Gram partials only involve the core's own
    rows: SU[a,bb] = U_a[rows_r]^T U_bb[rows_r] (upper triangle, emitted as
    levels complete so the scheduler can fill collective-wait gaps); host
    sums partials over cores.
  Host: with basis = levels 0..q_c-1 of each chain and the shift identity
  U_{t+1} = H U_t, build S0 = V^T V, S1 = V^T H V, S2 = (HV)^T (HV) from
  the SU blocks.  Both pencils (S1,S0) and (S2,S1) give Rayleigh-Ritz
  lower bounds for lambda_1; take the max (the (S2,S1) "harmonic" pencil
  is consistently tighter).  lambda = C * theta; answer = sum of top k.
"""

import numpy as np
import ml_dtypes

N_CORES = 8
M_ROWS = 8192
N_DIM = 4096
B_BLOCK = 96
QS = (4, 2)                  # Krylov applications per chain
CHAINS = len(QS)
CLIP_TH = 1e-5
WARMUP_MMS = 72

_NC_CACHE: dict = {}


def _build_nc(m_rows, n_dim, b, qs, n_cores, enable_asserts=False):
    import concourse.mybir as mybir
    import concourse.tile as tile
    from concourse import bacc
    from contextlib import ExitStack

    P = 128
    chains = len(qs)
    mloc = m_rows // n_cores   # 1024 rows of x per core
    ko_u = n_dim // P          # 32 k-tiles for U-matmul (and Ypart m-tiles)
    ko_p = mloc // P           # 8 k-tiles for Ypart-matmul (and U m-tiles)
    nlev = [q + 1 for q in qs]
    nblk = sum(nlev)
    bf = mybir.dt.bfloat16
    f32 = mybir.dt.float32

    nc = bacc.Bacc(
        "TRN2",
        target_bir_lowering=False,
        debug=False,
        enable_asserts=enable_asserts,
        num_devices=n_cores,
    )

    # xrl[m] holds x[rows of m-tile m, :]^T pi-major: [P(n), ko_u, P(m)]
    xrl = [nc.dram_tensor(f"xrl{m}", [P, ko_u, P], bf, kind="ExternalInput")
           for m in range(ko_p)]
    xrn = nc.dram_tensor("xrn", [P, ko_p, n_dim], bf, kind="ExternalInput")
    omega_l = [
        nc.dram_tensor(f"omega{c}", [P, ko_u, b], bf, kind="ExternalInput")
        for c in range(chains)
    ]
    p_out = nc.dram_tensor("p_out", [nblk * b, nblk * b], f32, kind="ExternalOutput")

    yp_in = [[nc.dram_tensor(f"ypi_{c}_{t}", [P, ko_u * b], bf) for t in range(qs[c])]
             for c in range(chains)]
    yp_out = [[nc.dram_tensor(f"ypo_{c}_{t}", [P, ko_u * b], bf, addr_space="Shared")
               for t in range(qs[c])] for c in range(chains)]

    rg = [list(range(n_cores))]

    with tile.TileContext(nc) as tc, ExitStack() as ctx:
        xpool = ctx.enter_context(tc.tile_pool(name="xin", bufs=1))
        ypool = ctx.enter_context(tc.tile_pool(name="yfull", bufs=1))
        yppool = ctx.enter_context(tc.tile_pool(name="ypart", bufs=1))
        slpool = ctx.enter_context(tc.tile_pool(name="slices", bufs=1))
        ppool = ctx.enter_context(tc.tile_pool(name="pout", bufs=3))
        # PSUM banks: 2 per chain rotating for U/Yp groups + 2 for SU/warmup
        pspool = ctx.enter_context(tc.tile_pool(name="ps", bufs=3, space="PSUM"))
        pspool2 = ctx.enter_context(tc.tile_pool(name="psp", bufs=2, space="PSUM"))

        # omegas first (tiny) on gpsimd; chain queues start on xrT right away
        dmae = [nc.sync, nc.scalar]  # per-chain DMA engine
        ycur = {}
        for c in range(chains):
            yf = ypool.tile([P, ko_u, b], bf, tag=f"yf{c}")
            nc.gpsimd.dma_start(yf[:], omega_l[c].ap())
            ycur[c] = yf

        # PE warmup burst during the x loads (HAM clock-gate ramp)
        wps = pspool2.tile([b, b], f32, tag="psp")
        for _ in range(WARMUP_MMS):
            nc.tensor.matmul(wps[:], ycur[0][:, 0, :], ycur[0][:, 0, :],
                             start=True, stop=True)

        # x loads: xrT first on ALL THREE queues (it gates the level-0
        # U-passes), then xn split 3/3/2 ko-tiles (needed ~14us later, for
        # the first Ypart).  All pieces are fully contiguous per partition
        # (strided column chunks proved ~1.5x slower per byte).
        xr_m = []
        xq = [nc.gpsimd, nc.sync, nc.scalar]
        for m in range(ko_p):
            t_ = xpool.tile([P, ko_u, P], bf, tag=f"xr{m}")
            xq[m % 3].dma_start(t_[:], xrl[m].ap())
            xr_m.append(t_)
        xn_t = xpool.tile([P, ko_p, n_dim], bf, tag="xn")
        for (e, lo, hi) in ((nc.gpsimd, 0, 3), (nc.sync, 3, 6),
                            (nc.scalar, 6, ko_p)):
            e.dma_start(xn_t[:, lo:hi, :], xrn.ap()[:, lo:hi, :])

        stored = []
        blocks = [(c, t) for c in range(chains) for t in range(qs[c] + 1)]
        bidx = {blk: i for i, blk in enumerate(blocks)}
        usl = {}

        pair_n = [0]
        pending = []
        eager = [False]

        def emit_p(z):
            # all pairs vs stored blocks + self: the (S2,S1) pencil needs the
            # top-level x top-level grams too, so no shift-only exclusion.
            for w in stored + [z]:
                a, bb = (w, z) if w < z else (z, w)
                ps = pspool2.tile([b, b], f32, tag="psp")
                ta = usl[blocks[a]]
                tb = usl[blocks[bb]]
                for ko in range(ko_p):
                    nc.tensor.matmul(
                        ps[:], ta[:, ko * b:(ko + 1) * b], tb[:, ko * b:(ko + 1) * b],
                        start=(ko == 0), stop=(ko == ko_p - 1),
                    )
                ob = ppool.tile([b, b], f32, tag="ob")
                nc.vector.tensor_copy(ob[:], ps[:])
                dmae[pair_n[0] % 2].dma_start(
                    p_out.ap()[a * b:(a + 1) * b, bb * b:(bb + 1) * b], ob[:]
                )
                pair_n[0] += 1
            stored.append(z)

        def u_mm(c, t):
            """usl[(c,t)] = x[rows_r,:] @ Y_t.  Groups of 2 m-tiles per PSUM
            buffer so the level-0 pass can start as soon as the first xrT
            m-tiles land (accumulation groups contiguous per bank region)."""
            us = slpool.tile([P, ko_p * b], bf, tag=f"usl{c}_{t}")
            for g in range(ko_p // 2):
                ps = pspool.tile([P, 2 * b], f32, tag=f"ps{c}")
                for m2 in range(2):
                    mo = g * 2 + m2
                    for ko in range(ko_u):
                        nc.tensor.matmul(
                            ps[:, m2 * b:(m2 + 1) * b],
                            xr_m[mo][:, ko, :],
                            ycur[c][:, ko, :],
                            start=(ko == 0),
                            stop=(ko == ko_u - 1),
                        )
                nc.vector.tensor_copy(us[:, g * 2 * b:(g + 1) * 2 * b], ps[:])
            usl[(c, t)] = us
            return us

        def step(c, t):
            """One Krylov application for chain c: U_t then Ypart -> AR."""
            eng = dmae[c % 2]
            us = u_mm(c, t)
            # Ypart = x[rows_r,:]^T @ U_t[rows_r]  [4096, b] pi-major
            yp = yppool.tile([P, ko_u * b], bf, tag=f"yp{c}")
            for g in range(ko_u // 4):
                ps = pspool.tile([P, 4 * b], f32, tag=f"ps{c}")
                for m2 in range(4):
                    mo = g * 4 + m2
                    for ko in range(ko_p):
                        nc.tensor.matmul(
                            ps[:, m2 * b:(m2 + 1) * b],
                            xn_t[:, ko, mo * P:(mo + 1) * P],
                            us[:, ko * b:(ko + 1) * b],
                            start=(ko == 0), stop=(ko == ko_p - 1),
                        )
                nc.vector.tensor_copy(yp[:, g * 4 * b:(g + 1) * 4 * b], ps[:])
                if g % 2 == 1:  # stream the finished half out right away
                    lo = (g - 1) * 4 * b
                    hi = (g + 1) * 4 * b
                    eng.dma_start(yp_in[c][t].ap()[:, lo:hi], yp[:, lo:hi])
            nc.gpsimd.collective_compute(
                "AllReduce", mybir.AluOpType.add, replica_groups=rg,
                ins=[yp_in[c][t].ap().opt()], outs=[yp_out[c][t].ap().opt()],
            )
            # AR result back to SBUF in 2 chunks on the chain's own queue
            # (cross-queue alternation makes the chains' DMA phases collide
            # mid-run; only the very last AR, when the other chain is done,
            # can safely use both queues)
            last_ar = (c, t) == (0, qs[0] - 1)
            yf = ypool.tile([P, ko_u, b], bf, tag=f"yf{c}")
            qv = ko_u // 4
            for i in range(4):
                # finer chunks let the U-pass start ~0.8us after AR end;
                # same-queue only (cross-queue alternation collides with the
                # other chain mid-run), except for the very last AR
                e_i = dmae[i % 2] if last_ar else eng
                e_i.dma_start(yf[:, i * qv:(i + 1) * qv, :],
                              yp_out[c][t].ap()[:, i * qv * b:(i + 1) * qv * b])
            ycur[c] = yf
            # mid-run cycles are PE-bound, so gram pairs are DEFERRED into
            # the tail AR windows (where the short chain has run out of
            # passes to hide the long chain's ARs) instead of inflating the
            # steady-state cadence.
            if eager[0]:
                for z in pending:
                    emit_p(z)
                pending.clear()
                emit_p(bidx[(c, t)])
            else:
                pending.append(bidx[(c, t)])

        # phase-shifted interleave of the chains.  A chain's final (shift)
        # U-pass only needs its own last AR, so emit it as soon as that AR
        # is one cycle old — it then covers the other chain's last AR and
        # only the longest chain's shift pass is exposed in the tail.
        done = set()

        def shift_and_flush(c, t):
            u_mm(c, t)
            # Emit just enough deferred pairs to cover THIS AR window; hold
            # the newest block back for the long chain's final AR window,
            # which has no other PE work to hide it.
            if not eager[0]:
                while len(pending) > 1:
                    emit_p(pending.pop(0))
                eager[0] = True
            emit_p(bidx[(c, t)])
            done.add((c, t))

        for t in range(max(qs)):
            for c in range(chains):
                if t < qs[c]:
                    step(c, t)
                elif t == qs[c] and (c, t) not in done:
                    shift_and_flush(c, t)
        for c in range(chains):
            if (c, qs[c]) not in done:
                shift_and_flush(c, qs[c])

    nc.compile()
    return nc


def _get_nc(cfg):
    if cfg not in _NC_CACHE:
        _NC_CACHE[cfg] = _build_nc(*cfg)
    return _NC_CACHE[cfg]


def _ritz_topk(S1, S0, k):
    """Top-k generalized eigenvalues of (S1, S0), f64, rank-guarded."""
    S1 = 0.5 * (S1 + S1.T)
    S0 = 0.5 * (S0 + S0.T)
    d = np.sqrt(np.clip(np.diag(S0), 0, None))
    d = np.where(d > 0, d, 1.0)
    dn = 1.0 / d
    S0n = S0 * dn[:, None] * dn[None, :]
    S1n = S1 * dn[:, None] * dn[None, :]
    w0, v0 = np.linalg.eigh(S0n)
    keep = w0 > (w0.max() * CLIP_TH)
    v = v0[:, keep] / np.sqrt(w0[keep])[None, :]
    m = v.T @ S1n @ v
    m = 0.5 * (m + m.T)
    ev = np.linalg.eigvalsh(m)
    ev = np.clip(ev, 0.0, None)
    return np.sort(ev)[::-1][:k]


def _host_solve(results, k, c_scale):
    b = B_BLOCK
    blocks = [(c, t) for c in range(CHAINS) for t in range(QS[c] + 1)]
    bidx = {blk: i for i, blk in enumerate(blocks)}
    nblk = len(blocks)
    P64 = np.zeros((nblk * b, nblk * b), dtype=np.float64)
    for r in results:
        p = r["p_out"].astype(np.float64)
        for a in range(nblk):
            for bb in range(a, nblk):
                blk = p[a * b:(a + 1) * b, bb * b:(bb + 1) * b]
                P64[a * b:(a + 1) * b, bb * b:(bb + 1) * b] += blk
                if bb != a:
                    P64[bb * b:(bb + 1) * b, a * b:(a + 1) * b] += blk.T

    def sub(alist, blist):
        rows = np.concatenate([np.arange(bidx[a] * b, (bidx[a] + 1) * b)
                               for a in alist])
        cols = np.concatenate([np.arange(bidx[bb] * b, (bidx[bb] + 1) * b)
                               for bb in blist])
        return P64[np.ix_(rows, cols)]

    bas = [(c, t) for c in range(CHAINS) for t in range(QS[c])]
    bas1 = [(c, t + 1) for (c, t) in bas]
    S0 = sub(bas, bas)
    S1 = sub(bas, bas1)
    S2 = sub(bas1, bas1)
    # Both pencils are Rayleigh-Ritz lower bounds on the top-k sum; the
    # (S2,S1) pencil is the H-weighted quotient and is tighter -> take max.
    vA = float(np.sum(_ritz_topk(S1, S0, k)))
    vB = float(np.sum(_ritz_topk(S2, S1, k)))
    return float(c_scale * max(vA, vB))


def _pi_major(a):
    """[K, m] -> [128, K//128, m] with out[pi, ko, m] = a[ko*128 + pi, m]."""
    K, m = a.shape
    return np.ascontiguousarray(a.reshape(K // 128, 128, m).transpose(1, 0, 2))


def _make_inputs(x_np, c_scale):
    bfd = ml_dtypes.bfloat16
    mloc = M_ROWS // N_CORES
    b = B_BLOCK
    P = 128
    xs = (x_np.astype(np.float64) / np.sqrt(c_scale)).astype(np.float32)
    xb = xs.astype(bfd)
    rng = np.random.default_rng(5)
    omegas = [
        rng.standard_normal((N_DIM, b)).astype(np.float32).astype(bfd)
        for _ in range(CHAINS)
    ]
    om_l = [_pi_major(om) for om in omegas]
    in_maps = []
    for r in range(N_CORES):
        xr = xb[r * mloc:(r + 1) * mloc, :]
        xrT = _pi_major(np.ascontiguousarray(xr.T))  # [P, ko_u, mloc]
        m = {"xrn": _pi_major(xr)}
        for mt in range(mloc // P):
            m[f"xrl{mt}"] = np.ascontiguousarray(
                xrT[:, :, mt * P:(mt + 1) * P])
        for c in range(CHAINS):
            m[f"omega{c}"] = om_l[c]
        in_maps.append(m)
    return in_maps


def _host_fallback(x_np, k_int):
    """Correct-but-slow host path, used only if the device result is bad."""
    x64 = x_np.astype(np.float64)
    blk = max(8, 2 * k_int)
    rng = np.random.default_rng(0)
    v = rng.standard_normal((x64.shape[1], blk))
    v, _ = np.linalg.qr(v)
    for _ in range(200):
        v, _ = np.linalg.qr(x64.T @ (x64 @ v))
    w = x64 @ v
    ev = np.linalg.eigvalsh(w.T @ w)
    return float(np.sum(np.sort(ev)[::-1][:k_int]))


def kernel(x, k):
    from concourse.bass_utils import run_bass_kernel_spmd

    x_np = np.asarray(x, dtype=np.float32)
    k_int = int(np.asarray(k))
    if k_int <= 0:
        return np.asarray(0.0, dtype=np.float32)

    try:
        v = x_np.ravel()
        fro2 = float(np.dot(v, v))
        c_scale = 3.0 * fro2 / N_DIM
        cfg = (M_ROWS, N_DIM, B_BLOCK, QS, N_CORES)
        nc = _get_nc(cfg)
        in_maps = _make_inputs(x_np, c_scale)
        res = run_bass_kernel_spmd(nc, in_maps, core_ids=list(range(N_CORES)))
        val = _host_solve(res.results, k_int, c_scale)
        if not np.isfinite(val) or val <= 0:
            raise FloatingPointError(f"bad device result {val}")
    except Exception:
        val = _host_fallback(x_np, k_int)
    return np.asarray(val, dtype=np.float32)


# revision 28
# speedup vs baseline: 1.0632x; 1.0632x over previous
"""Distributed BatchSpectralLoss kernel for Trainium2 (8 NeuronCores).

Computes sum of top-k squared singular values of x (= top-k eigenvalues of
the Gram matrix G = x^T x) for x of shape (8192, 4096), k small (k=1).

Algorithm — implicit block Krylov on x (G is never formed):
  Host: scale x by 1/sqrt(C) with C = 3*||x||_F^2/N so lamhat_1 = O(1) in
  bf16, and draw per-chain random start blocks Omega [4096, b].
  Device, per core r (bf16 matmuls, fp32 PSUM; r owns 1024 rows of x):
    SBUF-resident x slices: xrT = x[rows_r,:]^T (lhsT for U = x_r @ Y,
    loaded as 8 per-m-tile pieces striped over two DMA queues so the first
    U matmuls start ~8us in) and xrN = x[rows_r,:] (lhsT for the partial
    Y-update, striped over the vector+gpsimd queues).  A short PE warmup
    burst runs during the loads to beat the HAM cold-clock ramp.
    Per application t (q_c per chain, chains phase-shifted so one chain's
    matmuls hide the other's AllReduce; the gpsimd queue carries ONLY the
    collective triggers so ARs stream back-to-back):
      U_t[rows_r] = x[rows_r,:] @ Y_t           [1024, b]  (stays in SBUF)
      Ypart       = x[rows_r,:]^T @ U_t[rows_r] [4096, b]  (pi-major)
      AllReduce(add, bf16) Ypart -> Y_{t+1} full on every core.
    After the last level one extra U_{q_c} = x_r @ Y_{q_c}.
    The U_t sequence is a Krylov sequence of H = xhat xhat^T (same nonzero
    eigenvalues as Ghat), and its Gram partials only involve the core's own
    rows: SU[a,bb] = U_a[rows_r]^T U_bb[rows_r] (upper triangle, emitted as
    levels complete so the scheduler can fill collective-wait gaps); host
    sums partials over cores.
  Host: with basis = levels 0..q_c-1 of each chain and the shift identity
  U_{t+1} = H U_t, build S0 = V^T V, S1 = V^T H V, S2 = (HV)^T (HV) from
  the SU blocks.  Both pencils (S1,S0) and (S2,S1) give Rayleigh-Ritz
  lower bounds for lambda_1; take the max (the (S2,S1) "harmonic" pencil
  is consistently tighter).  lambda = C * theta; answer = sum of top k.
"""

import numpy as np
import ml_dtypes

N_CORES = 8
M_ROWS = 8192
N_DIM = 4096
B_BLOCK = 96
QS = (4, 2)                  # Krylov applications per chain
CHAINS = len(QS)
CLIP_TH = 1e-5
WARMUP_MMS = 72

_NC_CACHE: dict = {}


def _build_nc(m_rows, n_dim, b, qs, n_cores, enable_asserts=False):
    import concourse.mybir as mybir
    import concourse.tile as tile
    from concourse import bacc
    from contextlib import ExitStack

    P = 128
    chains = len(qs)
    mloc = m_rows // n_cores   # 1024 rows of x per core
    ko_u = n_dim // P          # 32 k-tiles for U-matmul (and Ypart m-tiles)
    ko_p = mloc // P           # 8 k-tiles for Ypart-matmul (and U m-tiles)
    nlev = [q + 1 for q in qs]
    nblk = sum(nlev)
    bf = mybir.dt.bfloat16
    f32 = mybir.dt.float32

    nc = bacc.Bacc(
        "TRN2",
        target_bir_lowering=False,
        debug=False,
        enable_asserts=enable_asserts,
        num_devices=n_cores,
    )

    # xrl[m] holds x[rows of m-tile m, :]^T pi-major: [P(n), ko_u, P(m)]
    xrl = [nc.dram_tensor(f"xrl{m}", [P, ko_u, P], bf, kind="ExternalInput")
           for m in range(ko_p)]
    xrn = nc.dram_tensor("xrn", [P, ko_p, n_dim], bf, kind="ExternalInput")
    omega_l = [
        nc.dram_tensor(f"omega{c}", [P, ko_u, b], bf, kind="ExternalInput")
        for c in range(chains)
    ]
    p_out = nc.dram_tensor("p_out", [nblk * b, nblk * b], f32, kind="ExternalOutput")

    yp_in = [[nc.dram_tensor(f"ypi_{c}_{t}", [P, ko_u * b], bf) for t in range(qs[c])]
             for c in range(chains)]
    yp_out = [[nc.dram_tensor(f"ypo_{c}_{t}", [P, ko_u * b], bf, addr_space="Shared")
               for t in range(qs[c])] for c in range(chains)]

    rg = [list(range(n_cores))]

    with tile.TileContext(nc) as tc, ExitStack() as ctx:
        xpool = ctx.enter_context(tc.tile_pool(name="xin", bufs=1))
        ypool = ctx.enter_context(tc.tile_pool(name="yfull", bufs=1))
        yppool = ctx.enter_context(tc.tile_pool(name="ypart", bufs=1))
        slpool = ctx.enter_context(tc.tile_pool(name="slices", bufs=1))
        ppool = ctx.enter_context(tc.tile_pool(name="pout", bufs=3))
        # PSUM banks: 2 per chain rotating for U/Yp groups + 2 for SU/warmup
        pspool = ctx.enter_context(tc.tile_pool(name="ps", bufs=3, space="PSUM"))
        pspool2 = ctx.enter_context(tc.tile_pool(name="psp", bufs=2, space="PSUM"))

        # omegas first (tiny) on gpsimd; chain queues start on xrT right away
        dmae = [nc.sync, nc.scalar]  # per-chain DMA engine
        ycur = {}
        for c in range(chains):
            yf = ypool.tile([P, ko_u, b], bf, tag=f"yf{c}")
            nc.gpsimd.dma_start(yf[:], omega_l[c].ap())
            ycur[c] = yf

        # PE warmup burst during the x loads (HAM clock-gate ramp)
        wps = pspool2.tile([b, b], f32, tag="psp")
        for _ in range(WARMUP_MMS):
            nc.tensor.matmul(wps[:], ycur[0][:, 0, :], ycur[0][:, 0, :],
                             start=True, stop=True)

        # x loads: xrT first on ALL THREE queues (it gates the level-0
        # U-passes), then xn split 3/3/2 ko-tiles (needed ~14us later, for
        # the first Ypart).  All pieces are fully contiguous per partition
        # (strided column chunks proved ~1.5x slower per byte).
        xr_m = []
        xq = [nc.gpsimd, nc.sync, nc.scalar]
        for m in range(ko_p):
            t_ = xpool.tile([P, ko_u, P], bf, tag=f"xr{m}")
            xq[m % 3].dma_start(t_[:], xrl[m].ap())
            xr_m.append(t_)
        xn_t = xpool.tile([P, ko_p, n_dim], bf, tag="xn")
        for (e, lo, hi) in ((nc.gpsimd, 0, 3), (nc.sync, 3, 6),
                            (nc.scalar, 6, ko_p)):
            e.dma_start(xn_t[:, lo:hi, :], xrn.ap()[:, lo:hi, :])

        stored = []
        blocks = [(c, t) for c in range(chains) for t in range(qs[c] + 1)]
        bidx = {blk: i for i, blk in enumerate(blocks)}
        usl = {}

        pair_n = [0]
        pending = []
        eager = [False]

        def emit_p(z):
            # all pairs vs stored blocks + self: the (S2,S1) pencil needs the
            # top-level x top-level grams too, so no shift-only exclusion.
            for w in stored + [z]:
                a, bb = (w, z) if w < z else (z, w)
                ps = pspool2.tile([b, b], f32, tag="psp")
                ta = usl[blocks[a]]
                tb = usl[blocks[bb]]
                for ko in range(ko_p):
                    nc.tensor.matmul(
                        ps[:], ta[:, ko * b:(ko + 1) * b], tb[:, ko * b:(ko + 1) * b],
                        start=(ko == 0), stop=(ko == ko_p - 1),
                    )
                ob = ppool.tile([b, b], f32, tag="ob")
                nc.vector.tensor_copy(ob[:], ps[:])
                dmae[pair_n[0] % 2].dma_start(
                    p_out.ap()[a * b:(a + 1) * b, bb * b:(bb + 1) * b], ob[:]
                )
                pair_n[0] += 1
            stored.append(z)

        def u_mm(c, t):
            """usl[(c,t)] = x[rows_r,:] @ Y_t.  Groups of 2 m-tiles per PSUM
            buffer so the level-0 pass can start as soon as the first xrT
            m-tiles land (accumulation groups contiguous per bank region)."""
            us = slpool.tile([P, ko_p * b], bf, tag=f"usl{c}_{t}")
            for g in range(ko_p // 2):
                ps = pspool.tile([P, 2 * b], f32, tag=f"ps{c}")
                for m2 in range(2):
                    mo = g * 2 + m2
                    for ko in range(ko_u):
                        nc.tensor.matmul(
                            ps[:, m2 * b:(m2 + 1) * b],
                            xr_m[mo][:, ko, :],
                            ycur[c][:, ko, :],
                            start=(ko == 0),
                            stop=(ko == ko_u - 1),
                        )
                nc.vector.tensor_copy(us[:, g * 2 * b:(g + 1) * 2 * b], ps[:])
            usl[(c, t)] = us
            return us

        def step(c, t):
            """One Krylov application for chain c: U_t then Ypart -> AR."""
            eng = dmae[c % 2]
            us = u_mm(c, t)
            # Ypart = x[rows_r,:]^T @ U_t[rows_r]  [4096, b] pi-major
            yp = yppool.tile([P, ko_u * b], bf, tag=f"yp{c}")
            for g in range(ko_u // 4):
                ps = pspool.tile([P, 4 * b], f32, tag=f"ps{c}")
                for m2 in range(4):
                    mo = g * 4 + m2
                    for ko in range(ko_p):
                        nc.tensor.matmul(
                            ps[:, m2 * b:(m2 + 1) * b],
                            xn_t[:, ko, mo * P:(mo + 1) * P],
                            us[:, ko * b:(ko + 1) * b],
                            start=(ko == 0), stop=(ko == ko_p - 1),
                        )
                nc.vector.tensor_copy(yp[:, g * 4 * b:(g + 1) * 4 * b], ps[:])
                if g % 2 == 1:  # stream the finished half out right away
                    lo = (g - 1) * 4 * b
                    hi = (g + 1) * 4 * b
                    eng.dma_start(yp_in[c][t].ap()[:, lo:hi], yp[:, lo:hi])
            nc.gpsimd.collective_compute(
                "AllReduce", mybir.AluOpType.add, replica_groups=rg,
                ins=[yp_in[c][t].ap().opt()], outs=[yp_out[c][t].ap().opt()],
            )
            # AR result back to SBUF in 2 chunks on the chain's own queue
            # (cross-queue alternation makes the chains' DMA phases collide
            # mid-run; only the very last AR, when the other chain is done,
            # can safely use both queues)
            last_ar = (c, t) == (0, qs[0] - 1)
            hw2 = (ko_u // 2) * b
            yf = ypool.tile([P, ko_u, b], bf, tag=f"yf{c}")
            e2 = dmae[1 - (c % 2)] if last_ar else eng
            eng.dma_start(yf[:, 0:ko_u // 2, :], yp_out[c][t].ap()[:, 0:hw2])
            e2.dma_start(yf[:, ko_u // 2:, :], yp_out[c][t].ap()[:, hw2:])
            ycur[c] = yf
            # mid-run cycles are PE-bound, so gram pairs are DEFERRED into
            # the tail AR windows (where the short chain has run out of
            # passes to hide the long chain's ARs) instead of inflating the
            # steady-state cadence.
            if eager[0]:
                emit_p(bidx[(c, t)])
            else:
                pending.append(bidx[(c, t)])

        # phase-shifted interleave of the chains.  A chain's final (shift)
        # U-pass only needs its own last AR, so emit it as soon as that AR
        # is one cycle old — it then covers the other chain's last AR and
        # only the longest chain's shift pass is exposed in the tail.
        done = set()

        def shift_and_flush(c, t):
            u_mm(c, t)
            for z in pending:
                emit_p(z)
            pending.clear()
            eager[0] = True
            emit_p(bidx[(c, t)])
            done.add((c, t))

        for t in range(max(qs)):
            for c in range(chains):
                if t < qs[c]:
                    step(c, t)
                elif t == qs[c] and (c, t) not in done:
                    shift_and_flush(c, t)
        for c in range(chains):
            if (c, qs[c]) not in done:
                shift_and_flush(c, qs[c])

    nc.compile()
    return nc


def _get_nc(cfg):
    if cfg not in _NC_CACHE:
        _NC_CACHE[cfg] = _build_nc(*cfg)
    return _NC_CACHE[cfg]


def _ritz_topk(S1, S0, k):
    """Top-k generalized eigenvalues of (S1, S0), f64, rank-guarded."""
    S1 = 0.5 * (S1 + S1.T)
    S0 = 0.5 * (S0 + S0.T)
    d = np.sqrt(np.clip(np.diag(S0), 0, None))
    d = np.where(d > 0, d, 1.0)
    dn = 1.0 / d
    S0n = S0 * dn[:, None] * dn[None, :]
    S1n = S1 * dn[:, None] * dn[None, :]
    w0, v0 = np.linalg.eigh(S0n)
    keep = w0 > (w0.max() * CLIP_TH)
    v = v0[:, keep] / np.sqrt(w0[keep])[None, :]
    m = v.T @ S1n @ v
    m = 0.5 * (m + m.T)
    ev = np.linalg.eigvalsh(m)
    ev = np.clip(ev, 0.0, None)
    return np.sort(ev)[::-1][:k]


def _host_solve(results, k, c_scale):
    b = B_BLOCK
    blocks = [(c, t) for c in range(CHAINS) for t in range(QS[c] + 1)]
    bidx = {blk: i for i, blk in enumerate(blocks)}
    nblk = len(blocks)
    P64 = np.zeros((nblk * b, nblk * b), dtype=np.float64)
    for r in results:
        p = r["p_out"].astype(np.float64)
        for a in range(nblk):
            for bb in range(a, nblk):
                blk = p[a * b:(a + 1) * b, bb * b:(bb + 1) * b]
                P64[a * b:(a + 1) * b, bb * b:(bb + 1) * b] += blk
                if bb != a:
                    P64[bb * b:(bb + 1) * b, a * b:(a + 1) * b] += blk.T

    def sub(alist, blist):
        rows = np.concatenate([np.arange(bidx[a] * b, (bidx[a] + 1) * b)
                               for a in alist])
        cols = np.concatenate([np.arange(bidx[bb] * b, (bidx[bb] + 1) * b)
                               for bb in blist])
        return P64[np.ix_(rows, cols)]

    bas = [(c, t) for c in range(CHAINS) for t in range(QS[c])]
    bas1 = [(c, t + 1) for (c, t) in bas]
    S0 = sub(bas, bas)
    S1 = sub(bas, bas1)
    S2 = sub(bas1, bas1)
    # Both pencils are Rayleigh-Ritz lower bounds on the top-k sum; the
    # (S2,S1) pencil is the H-weighted quotient and is tighter -> take max.
    vA = float(np.sum(_ritz_topk(S1, S0, k)))
    vB = float(np.sum(_ritz_topk(S2, S1, k)))
    return float(c_scale * max(vA, vB))


def _pi_major(a):
    """[K, m] -> [128, K//128, m] with out[pi, ko, m] = a[ko*128 + pi, m]."""
    K, m = a.shape
    return np.ascontiguousarray(a.reshape(K // 128, 128, m).transpose(1, 0, 2))


def _make_inputs(x_np, c_scale):
    bfd = ml_dtypes.bfloat16
    mloc = M_ROWS // N_CORES
    b = B_BLOCK
    P = 128
    xs = (x_np.astype(np.float64) / np.sqrt(c_scale)).astype(np.float32)
    xb = xs.astype(bfd)
    rng = np.random.default_rng(5)
    omegas = [
        rng.standard_normal((N_DIM, b)).astype(np.float32).astype(bfd)
        for _ in range(CHAINS)
    ]
    om_l = [_pi_major(om) for om in omegas]
    in_maps = []
    for r in range(N_CORES):
        xr = xb[r * mloc:(r + 1) * mloc, :]
        xrT = _pi_major(np.ascontiguousarray(xr.T))  # [P, ko_u, mloc]
        m = {"xrn": _pi_major(xr)}
        for mt in range(mloc // P):
            m[f"xrl{mt}"] = np.ascontiguousarray(
                xrT[:, :, mt * P:(mt + 1) * P])
        for c in range(CHAINS):
            m[f"omega{c}"] = om_l[c]
        in_maps.append(m)
    return in_maps


def _host_fallback(x_np, k_int):
    """Correct-but-slow host path, used only if the device result is bad."""
    x64 = x_np.astype(np.float64)
    blk = max(8, 2 * k_int)
    rng = np.random.default_rng(0)
    v = rng.standard_normal((x64.shape[1], blk))
    v, _ = np.linalg.qr(v)
    for _ in range(200):
        v, _ = np.linalg.qr(x64.T @ (x64 @ v))
    w = x64 @ v
    ev = np.linalg.eigvalsh(w.T @ w)
    return float(np.sum(np.sort(ev)[::-1][:k_int]))


def kernel(x, k):
    from concourse.bass_utils import run_bass_kernel_spmd

    x_np = np.asarray(x, dtype=np.float32)
    k_int = int(np.asarray(k))
    if k_int <= 0:
        return np.asarray(0.0, dtype=np.float32)

    try:
        v = x_np.ravel()
        fro2 = float(np.dot(v, v))
        c_scale = 3.0 * fro2 / N_DIM
        cfg = (M_ROWS, N_DIM, B_BLOCK, QS, N_CORES)
        nc = _get_nc(cfg)
        in_maps = _make_inputs(x_np, c_scale)
        res = run_bass_kernel_spmd(nc, in_maps, core_ids=list(range(N_CORES)))
        val = _host_solve(res.results, k_int, c_scale)
        if not np.isfinite(val) or val <= 0:
            raise FloatingPointError(f"bad device result {val}")
    except Exception:
        val = _host_fallback(x_np, k_int)
    return np.asarray(val, dtype=np.float32)
